# revision 8
# baseline (speedup 1.0000x reference)
"""Action-conditioned transformer forward on 8 TRN2 NeuronCores.

Strategy: pure data-parallel over batch (B=8 -> 1 element per core, zero
collectives). Residual stream kept transposed in SBUF (x^T [768, 1024] f32),
so every matmul contracts along partitions with no PE transposes. All matmuls
in bf16 (1 cyc/row); stats/PSUM/residual in f32. Softmax without
max-subtraction (logits tiny; exp evaluated in f32 on ACT), denominators via a
ones-column appended to V and reciprocal_approx_fast.
"""
import numpy as np
import ml_dtypes
from contextlib import ExitStack

import concourse.bass as bass
import concourse.bacc as bacc
import concourse.tile as tile
import concourse.mybir as mybir
from concourse.bass_utils import run_bass_kernel_spmd

F32 = mybir.dt.float32
BF16 = mybir.dt.bfloat16
I32 = mybir.dt.int32
AF = mybir.ActivationFunctionType
ALU = mybir.AluOpType
BF = ml_dtypes.bfloat16

B, S, D, H, L = 8, 1024, 768, 12, 8
DH, V, COND, FF = 64, 1024, 256, 3072
NT, ND, NV, NF = S // 128, D // 128, V // 128, FF // 128  # 8, 6, 8, 24
FQ = FF // 4  # 768
EPS = 1e-5
INV_SQRT_DH = 1.0 / 8.0

_CACHE = {}


def _bcast_ap(row_ap, nparts):
    """Free-dim step-0 replication source AP from a [1, N] row at any partition."""
    return bass.AP(row_ap.tensor, row_ap.offset,
                   [row_ap.ap[0], [0, nparts]] + row_ap.ap[1:])


def _build():
    nc = bacc.Bacc("TRN2", target_bir_lowering=False, debug=False)

    tok_e = nc.dram_tensor("tok", [1, S], F32, kind="ExternalInput")
    av_e = nc.dram_tensor("avec", [128, 2], BF16, kind="ExternalInput")
    emb_e = nc.dram_tensor("embw", [128, NV, D], BF16, kind="ExternalInput")
    pet_e = nc.dram_tensor("pet", [128, ND, S], F32, kind="ExternalInput")
    lnw_e = nc.dram_tensor("lnw", [L, 128, 8, D], BF16, kind="ExternalInput")
    lnf_e = nc.dram_tensor("lnf", [128, 4, D], BF16, kind="ExternalInput")
    wqkv_e = nc.dram_tensor("wqkv", [L, 128, 18, DH], BF16, kind="ExternalInput")
    wo_e = nc.dram_tensor("wo", [L, 64, H, D], BF16, kind="ExternalInput")
    bo_e = nc.dram_tensor("bo", [L, 1, D], BF16, kind="ExternalInput")
    w1_e = nc.dram_tensor("w1", [L, 128, ND, FF], BF16, kind="ExternalInput")
    b1_e = nc.dram_tensor("b1c", [L, 128, NF], F32, kind="ExternalInput")
    w2_e = nc.dram_tensor("w2", [L, 128, NF, D], BF16, kind="ExternalInput")
    b2_e = nc.dram_tensor("b2", [L, 1, D], BF16, kind="ExternalInput")
    wout_e = nc.dram_tensor("wout", [128, ND, V], BF16, kind="ExternalInput")
    bout_e = nc.dram_tensor("bout", [1, V], BF16, kind="ExternalInput")
    out_e = nc.dram_tensor("out", [S, V], F32, kind="ExternalOutput")

    with tile.TileContext(nc) as tc, ExitStack() as ctx:
        con = ctx.enter_context(tc.tile_pool(name="con", bufs=1))
        xp = ctx.enter_context(tc.tile_pool(name="xp", bufs=1))
        hp = ctx.enter_context(tc.tile_pool(name="hp", bufs=1))
        big = ctx.enter_context(tc.tile_pool(name="big", bufs=1))
        wp = ctx.enter_context(tc.tile_pool(name="wp", bufs=1))
        w12 = ctx.enter_context(tc.tile_pool(name="w12", bufs=2))
        sc = ctx.enter_context(tc.tile_pool(name="sc", bufs=2))
        rp = ctx.enter_context(tc.tile_pool(name="rp", bufs=2))
        ps = ctx.enter_context(tc.tile_pool(name="ps", bufs=8, space="PSUM"))

        # ---- constants
        ones_c = con.tile([128, 1], BF16, tag="ones_c")
        nc.vector.memset(ones_c[:], 1.0)
        ones_r = con.tile([1, 512], BF16, tag="ones_r")
        nc.vector.memset(ones_r[:], 1.0)
        ones_r128 = con.tile([1, 128], BF16, tag="ones_r128")
        nc.vector.memset(ones_r128[:], 1.0)
        eps_t = con.tile([1, 1], F32, tag="eps_t")
        nc.vector.memset(eps_t[:], EPS)
        vidx = con.tile([128, NV], F32, tag="vidx")
        vidx_i = con.tile([128, 1], I32, tag="vidx_i")
        nc.gpsimd.iota(vidx_i[:], [[0, 1]], base=0, channel_multiplier=1)
        vidx_f = con.tile([128, 1], F32, tag="vidx_f")
        nc.vector.tensor_copy(vidx_f[:], vidx_i[:])
        for vc in range(NV):
            nc.vector.tensor_scalar(vidx[:, vc:vc + 1], vidx_f[:], float(vc * 128), None, ALU.add)
        avec = con.tile([128, 2], BF16, tag="avec")
        nc.sync.dma_start(avec[:], av_e.ap()[:])

        # residual x^T [128, ND*S] f32 and h^T bf16
        xT = xp.tile([128, ND * S], F32, tag="xT")
        hT = hp.tile([128, ND * S], BF16, tag="hT")

        # ---- embedding: x^T = emb^T @ onehot(tok) + pe^T  (scoped pools, 2 dc-groups)
        with tc.tile_pool(name="ep1", bufs=2) as ep1:
            tok_f = ep1.tile([1, S], F32, tag="tok_f", bufs=1)
            nc.sync.dma_start(tok_f[:], tok_e.ap()[:])
            tokrep = ep1.tile([128, S], F32, tag="tokrep", bufs=1)
            nc.gpsimd.partition_broadcast(tokrep[:], tok_f[:])
            for g in range(2):
                pss = [ps.tile([128, 512], F32, tag="ps", name=f"pss{g}_{i}") for i in range(6)]
                for vc in range(NV):
                    oh = ep1.tile([128, S], BF16, tag="oh", name=f"oh{g}_{vc}")
                    nc.vector.tensor_scalar(oh[:], tokrep[:], vidx[:, vc:vc + 1], None, ALU.is_equal)
                    embq = ep1.tile([128, D], BF16, tag="embq", name=f"embq{g}_{vc}")
                    nc.sync.dma_start(embq[:], emb_e.ap()[:, vc, :])
                    for dc3 in range(3):
                        dc = g * 3 + dc3
                        for nh in range(2):
                            nc.tensor.matmul(pss[dc3 * 2 + nh][:],
                                             embq[:, dc * 128: dc * 128 + 128],
                                             oh[:, nh * 512: nh * 512 + 512],
                                             start=(vc == 0), stop=(vc == NV - 1))
                for dc3 in range(3):
                    dc = g * 3 + dc3
                    pet = ep1.tile([128, S], F32, tag="pet", bufs=1, name=f"pet{dc}")
                    nc.sync.dma_start(pet[:], pet_e.ap()[:, dc, :])
                    for nh in range(2):
                        nc.vector.tensor_tensor(xT[:, dc * S + nh * 512: dc * S + nh * 512 + 512],
                                                pss[dc3 * 2 + nh][:], pet[:, nh * 512: nh * 512 + 512], ALU.add)

        def cond_vectors(lnw_tile, nvec, gc_tag):
            """gcols [128, ND*nvec] f32: column (dc*nvec + m) = vec m, d-chunk dc."""
            gcols = sc.tile([128, ND * nvec], F32, tag=gc_tag, name=gc_tag)
            for dc in range(ND):
                gv = ps.tile([128, nvec], F32, tag="ps", name=f"gv{dc}")
                for m in range(nvec):
                    for kc in range(2):
                        nc.tensor.matmul(gv[:, m:m + 1],
                                         lnw_tile[:, (m * 2 + kc) * D + dc * 128: (m * 2 + kc) * D + dc * 128 + 128],
                                         avec[:, kc:kc + 1],
                                         start=(kc == 0), stop=(kc == 1))
                nc.scalar.copy(gcols[:, dc * nvec: dc * nvec + nvec], gv[:])
            return gcols

        def cond_ln(src, dst, gcols, gi, bi, nvec):
            """dst (bf16) = condLN(src f32), gain/bias from gcols cols dc*nvec+{gi,bi}."""
            xbs, x2s = [], []
            for dc in range(ND):
                xb = sc.tile([128, S], BF16, tag="xb", name=f"xb{dc}")
                nc.vector.tensor_scalar(xb[:], src[:, dc * S:(dc + 1) * S], 1.0, None, ALU.mult)
                x2 = sc.tile([128, S], BF16, tag="x2", name=f"x2{dc}")
                nc.vector.tensor_tensor(x2[:], xb[:], xb[:], ALU.mult)
                xbs.append(xb)
                x2s.append(x2)
            sx = [ps.tile([1, 512], F32, tag="ps", name=f"sx{i}") for i in range(2)]
            sxx = [ps.tile([1, 512], F32, tag="ps", name=f"sxx{i}") for i in range(2)]
            for nh in range(2):
                for dc in range(ND):
                    nc.tensor.matmul(sx[nh][:], ones_c[:], xbs[dc][:, nh * 512: nh * 512 + 512],
                                     start=(dc == 0), stop=(dc == ND - 1))
                for dc in range(ND):
                    nc.tensor.matmul(sxx[nh][:], ones_c[:], x2s[dc][:, nh * 512: nh * 512 + 512],
                                     start=(dc == 0), stop=(dc == ND - 1))
            mu = sc.tile([1, S], F32, tag="rowA", bufs=1, name="mu")
            var = sc.tile([1, S], F32, tag="rowB", bufs=1, name="var")
            for nh in range(2):
                sl = slice(nh * 512, nh * 512 + 512)
                nc.vector.tensor_scalar(mu[:, sl], sx[nh][:], 1.0 / D, None, ALU.mult)
                nc.vector.tensor_scalar(var[:, sl], sxx[nh][:], 1.0 / D, None, ALU.mult)
            murep = rp.tile([128, S], F32, tag="rep", name="murep")
            nc.gpsimd.partition_broadcast(murep[:], mu[:])
            tmp = sc.tile([1, S], F32, tag="rowC", bufs=1, name="mu2")
            nc.vector.tensor_tensor(tmp[:], mu[:], mu[:], ALU.mult)
            nc.vector.tensor_tensor(var[:], var[:], tmp[:], ALU.subtract)
            sd = sc.tile([1, S], F32, tag="rowC", bufs=1, name="sd")
            nc.scalar.activation(sd[:], var[:], AF.Sqrt, bias=eps_t[:], scale=1.0)
            rr = sc.tile([1, S], F32, tag="rowA", bufs=1, name="rr")
            nc.vector.reciprocal_approx_fast(rr[:], sd[:])
            rrep = rp.tile([128, S], F32, tag="rep", name="rrep")
            nc.gpsimd.partition_broadcast(rrep[:], rr[:])
            for dc in range(ND):
                t = sc.tile([128, S], BF16, tag="x2", name=f"lnt{dc}")
                nc.vector.tensor_tensor(t[:], src[:, dc * S:(dc + 1) * S], murep[:], ALU.subtract)
                t2 = sc.tile([128, S], BF16, tag="xb", name=f"lnt2_{dc}")
                nc.vector.scalar_tensor_tensor(t2[:], t[:], gcols[:, dc * nvec + gi: dc * nvec + gi + 1],
                                               rrep[:], ALU.mult, ALU.mult)
                nc.vector.tensor_scalar(dst[:, dc * S:(dc + 1) * S], t2[:],
                                        gcols[:, dc * nvec + bi: dc * nvec + bi + 1], None, ALU.add)

        # =============== layers ===============
        for l in range(L):
            lnw = wp.tile([128, 8 * D], BF16, tag="lnw", name=f"lnw{l}")
            nc.sync.dma_start(lnw[:], lnw_e.ap()[l])
            gcols = cond_vectors(lnw, 4, "gcols")

            wqkv = wp.tile([128, 18 * DH], BF16, tag="wqkv", name=f"wqkv{l}")
            nc.sync.dma_start(wqkv[:], wqkv_e.ap()[l])
            wo = wp.tile([64, H * D], BF16, tag="wo", name=f"wo{l}")
            nc.sync.dma_start(wo[:], wo_e.ap()[l])
            bo = wp.tile([1, D], BF16, tag="bo", name=f"bo{l}")
            nc.sync.dma_start(bo[:], bo_e.ap()[l])
            b1c = wp.tile([128, NF], F32, tag="b1c", name=f"b1c{l}")
            nc.sync.dma_start(b1c[:], b1_e.ap()[l])
            b2 = wp.tile([1, D], BF16, tag="b2", name=f"b2{l}")
            nc.sync.dma_start(b2[:], b2_e.ap()[l])

            # ---- LN1 -> hT
            cond_ln(xT, hT, gcols, 0, 1, 4)

            # ---- attention
            o_all = big.tile([64, H * S], BF16, tag="big", name=f"oall{l}")
            sall = sc.tile([12, S], F32, tag="sall", bufs=1, name=f"sall{l}")
            for h in range(H):
                base = 64 * (h % 2)
                hsl = hT[base:base + 64, (h // 2) * S: (h // 2) * S + S]

                def wsl(s):
                    c = (s * 6 + h // 2) * DH
                    return wqkv[base:base + 64, c:c + DH]

                qb = sc.tile([64, S], BF16, tag="qb", name=f"qb{h}")
                kb = sc.tile([64, S], BF16, tag="kb", name=f"kb{h}")
                for which, dstt in ((0, qb), (1, kb)):
                    for nh in range(2):
                        pq = ps.tile([64, 512], F32, tag="ps", name=f"pq{h}_{which}_{nh}")
                        nc.tensor.matmul(pq[:], wsl(which), hsl[:, nh * 512: nh * 512 + 512],
                                         start=True, stop=True)
                        nc.vector.tensor_copy(dstt[:, nh * 512: nh * 512 + 512], pq[:])
                vt = sc.tile([128, NT * 65], BF16, tag="vt", name=f"vt{h}")
                nc.vector.memset(vt[:], 1.0)
                for tc8 in range(NT):
                    pv = ps.tile([128, 64], F32, tag="ps", name=f"pv{h}_{tc8}")
                    nc.tensor.matmul(pv[:], hsl[:, tc8 * 128: tc8 * 128 + 128], wsl(2),
                                     start=True, stop=True)
                    nc.vector.tensor_copy(vt[:, tc8 * 65: tc8 * 65 + 64], pv[:])
                ops_ = [ps.tile([65, 512], F32, tag="ps", name=f"oaug{h}_{i}") for i in range(2)]
                for kc in range(NT):
                    pt = sc.tile([128, S], BF16, tag="pt", name=f"pt{h}_{kc}")
                    for nh in range(2):
                        pl = ps.tile([128, 512], F32, tag="ps", name=f"pl{h}_{kc}_{nh}")
                        nc.tensor.matmul(pl[:], kb[:, kc * 128: kc * 128 + 128],
                                         qb[:, nh * 512: nh * 512 + 512], start=True, stop=True)
                        nc.scalar.activation(pt[:, nh * 512: nh * 512 + 512], pl[:],
                                             AF.Exp, bias=0.0, scale=INV_SQRT_DH)
                    for nh in range(2):
                        nc.tensor.matmul(ops_[nh][:], vt[:, kc * 65: kc * 65 + 65],
                                         pt[:, nh * 512: nh * 512 + 512],
                                         start=(kc == 0), stop=(kc == NT - 1))
                s64 = sc.tile([65, S], F32, tag="s64", bufs=1, name=f"s64_{h}")
                for nh in range(2):
                    sl = slice(nh * 512, nh * 512 + 512)
                    nc.scalar.copy(o_all[:, h * S + nh * 512: h * S + nh * 512 + 512], ops_[nh][0:64, :])
                    nc.scalar.copy(s64[64:65, sl], ops_[nh][64:65, :])
                nc.sync.dma_start(sall[h:h + 1, :], s64[64:65, :])
            rall = sall
            nc.vector.reciprocal_approx_fast(rall[:], sall[:])
            for h in range(H):
                rrh = rp.tile([64, S], F32, tag="rep", name=f"rrh{h}")
                nc.sync.dma_start(rrh[:], _bcast_ap(rall[h:h + 1, :], 64))
                nc.vector.tensor_tensor(o_all[:, h * S:(h + 1) * S],
                                        o_all[:, h * S:(h + 1) * S], rrh[:], ALU.mult)
            # y^T accumulate + residual
            for th in range(2):
                for dc in range(ND):
                    py = ps.tile([128, 512], F32, tag="ps", name=f"py{th}_{dc}")
                    for h in range(H):
                        nc.tensor.matmul(py[:], wo[:, h * D + dc * 128: h * D + dc * 128 + 128],
                                         o_all[:, h * S + th * 512: h * S + th * 512 + 512],
                                         start=(h == 0), stop=False)
                    nc.tensor.matmul(py[:], bo[:, dc * 128: dc * 128 + 128], ones_r[:],
                                     start=False, stop=True)
                    sl = slice(dc * S + th * 512, dc * S + th * 512 + 512)
                    nc.vector.tensor_tensor(xT[:, sl], xT[:, sl], py[:], ALU.add)

            # ---- LN2 -> hT
            cond_ln(xT, hT, gcols, 2, 3, 4)

            # ---- FFN in four hidden quarters
            for qq in range(4):
                w1q = w12.tile([128, ND * FQ], BF16, tag="w1q", name=f"w1q{l}_{qq}")
                nc.sync.dma_start(w1q[:], w1_e.ap()[l][:, :, qq * FQ:(qq + 1) * FQ])
                gelu = big.tile([128, 6 * S], BF16, tag="big", name=f"gelu{l}_{qq}")
                for mc in range(6):
                    gmc = qq * 6 + mc
                    for nh in range(2):
                        pf = ps.tile([128, 512], F32, tag="ps", name=f"pf{qq}_{mc}_{nh}")
                        for kc in range(ND):
                            nc.tensor.matmul(pf[:],
                                             w1q[:, kc * FQ + mc * 128: kc * FQ + mc * 128 + 128],
                                             hT[:, kc * S + nh * 512: kc * S + nh * 512 + 512],
                                             start=(kc == 0), stop=(kc == ND - 1))
                        nc.scalar.activation(gelu[:, mc * S + nh * 512: mc * S + nh * 512 + 512],
                                             pf[:], AF.Gelu_apprx_tanh,
                                             bias=b1c[:, gmc:gmc + 1], scale=1.0)
                w2q = w12.tile([128, 6 * D], BF16, tag="w2q", name=f"w2q{l}_{qq}")
                nc.sync.dma_start(w2q[:], w2_e.ap()[l][:, qq * 6:(qq + 1) * 6, :])
                for th in range(2):
                    for dc in range(ND):
                        py = ps.tile([128, 512], F32, tag="ps", name=f"py2_{qq}_{th}_{dc}")
                        for kc in range(6):
                            nc.tensor.matmul(py[:], w2q[:, kc * D + dc * 128: kc * D + dc * 128 + 128],
                                             gelu[:, kc * S + th * 512: kc * S + th * 512 + 512],
                                             start=(kc == 0), stop=(kc == 5 and qq != 3))
                        if qq == 3:
                            nc.tensor.matmul(py[:], b2[:, dc * 128: dc * 128 + 128], ones_r[:],
                                             start=False, stop=True)
                        sl = slice(dc * S + th * 512, dc * S + th * 512 + 512)
                        nc.vector.tensor_tensor(xT[:, sl], xT[:, sl], py[:], ALU.add)

        # =============== final LN + unembed ===============
        lnf = wp.tile([128, 4 * D], BF16, tag="lnw", name="lnf")
        nc.sync.dma_start(lnf[:], lnf_e.ap()[:])
        gcf = cond_vectors(lnf, 2, "gcf")
        cond_ln(xT, hT, gcf, 0, 1, 2)

        woutt = big.tile([128, ND * V], BF16, tag="big", name="woutt")
        nc.sync.dma_start(woutt[:], wout_e.ap()[:])
        boutt = wp.tile([1, V], BF16, tag="bo", name="boutt")
        nc.sync.dma_start(boutt[:], bout_e.ap()[:])
        for tc8 in range(NT):
            osb = sc.tile([128, V], F32, tag="s64", bufs=1, name=f"osb{tc8}")
            for nh in range(2):
                po = ps.tile([128, 512], F32, tag="ps", name=f"po{tc8}_{nh}")
                for dc in range(ND):
                    nc.tensor.matmul(po[:], hT[:, dc * S + tc8 * 128: dc * S + tc8 * 128 + 128],
                                     woutt[:, dc * V + nh * 512: dc * V + nh * 512 + 512],
                                     start=(dc == 0), stop=False)
                nc.tensor.matmul(po[:], ones_r128[:], boutt[:, nh * 512: nh * 512 + 512],
                                 start=False, stop=True)
                nc.vector.tensor_copy(osb[:, nh * 512: nh * 512 + 512], po[:])
            nc.sync.dma_start(out_e.ap()[tc8 * 128:(tc8 + 1) * 128, :], osb[:])

    nc.compile()
    return nc


def _sinusoidal_pe():
    pos = np.arange(S, dtype=np.float32)[:, None]
    div = np.exp(np.arange(0, D, 2, dtype=np.float32) * (-np.log(10000.0) / D))
    pe = np.zeros((S, D), dtype=np.float32)
    pe[:, 0::2] = np.sin(pos * div)
    pe[:, 1::2] = np.cos(pos * div)
    return pe


def _pack_weights(ins):
    """Host-side packing into partition-major SBUF images."""
    w = {}
    emb = np.asarray(ins["emb"], np.float32)
    w["embw"] = np.ascontiguousarray(emb.reshape(NV, 128, D).transpose(1, 0, 2)).astype(BF)
    w["pet"] = np.ascontiguousarray(
        (_sinusoidal_pe().T).reshape(ND, 128, S).transpose(1, 0, 2)).astype(np.float32)

    lnw = np.zeros((L, 128, 8, D), np.float32)
    for l in range(L):
        for m, name in enumerate(["ln1_g", "ln1_b", "ln2_g", "ln2_b"]):
            mat = np.asarray(ins[name], np.float32)[l]  # [COND, D]
            lnw[l, :, m * 2 + 0, :] = mat[0:128]
            lnw[l, :, m * 2 + 1, :] = mat[128:256]
    w["lnw"] = lnw.astype(BF)
    lnf = np.zeros((128, 4, D), np.float32)
    for m, name in enumerate(["lnf_g", "lnf_b"]):
        mat = np.asarray(ins[name], np.float32)
        lnf[:, m * 2 + 0, :] = mat[0:128]
        lnf[:, m * 2 + 1, :] = mat[128:256]
    w["lnf"] = lnf.astype(BF)

    wqkv = np.zeros((L, 128, 18, DH), np.float32)
    for l in range(L):
        for s, name in enumerate(["Wq", "Wk", "Wv"]):
            mat = np.asarray(ins[name], np.float32)[l]  # [H, DH, DH]
            for h in range(H):
                wqkv[l, 64 * (h % 2):64 * (h % 2) + 64, s * 6 + h // 2, :] = mat[h]
    w["wqkv"] = wqkv.astype(BF)

    wo = np.zeros((L, 64, H, D), np.float32)
    for l in range(L):
        mat = np.asarray(ins["Wo"], np.float32)[l]
        for h in range(H):
            wo[l, :, h, :] = mat[64 * h:64 * h + 64, :]
    w["wo"] = wo.astype(BF)
    w["bo"] = np.asarray(ins["bo"], np.float32).reshape(L, 1, D).astype(BF)
    w["w1"] = np.ascontiguousarray(
        np.asarray(ins["W1"], np.float32).reshape(L, ND, 128, FF).transpose(0, 2, 1, 3)).astype(BF)
    w["b1c"] = np.ascontiguousarray(
        np.asarray(ins["b1"], np.float32).reshape(L, NF, 128).transpose(0, 2, 1)).astype(np.float32)
    w["w2"] = np.ascontiguousarray(
        np.asarray(ins["W2"], np.float32).reshape(L, NF, 128, D).transpose(0, 2, 1, 3)).astype(BF)
    w["b2"] = np.asarray(ins["b2"], np.float32).reshape(L, 1, D).astype(BF)
    w["wout"] = np.ascontiguousarray(
        np.asarray(ins["Wout"], np.float32).reshape(ND, 128, V).transpose(1, 0, 2)).astype(BF)
    w["bout"] = np.asarray(ins["bout"], np.float32).reshape(1, V).astype(BF)
    return w


def kernel(**inputs):
    if "nc" not in _CACHE:
        _CACHE["nc"] = _build()
    nc = _CACHE["nc"]

    w = _pack_weights(inputs)
    tokens = np.asarray(inputs["tokens"]).astype(np.int32)      # [B, S]
    actions = np.asarray(inputs["actions"]).astype(np.int64)    # [B, NACT]
    act_emb = np.asarray(inputs["act_emb"], np.float32)         # [NCOND, DPA]

    in_maps = []
    for b in range(B):
        a = act_emb[actions[b]].reshape(COND).astype(np.float32)  # [256]
        av = np.ascontiguousarray(a.reshape(2, 128).T).astype(BF)  # [128, 2]
        m = {"tok": tokens[b:b + 1].astype(np.float32), "avec": av}
        m.update(w)
        in_maps.append(m)

    res = run_bass_kernel_spmd(nc, in_maps, core_ids=list(range(B)))
    out = np.stack([res.results[b]["out"] for b in range(B)], axis=0)
    return out.astype(np.float32)


# revision 13
# speedup vs baseline: 1.1034x; 1.1034x over previous
"""Action-conditioned transformer forward on 8 TRN2 NeuronCores.

Strategy: pure data-parallel over batch (B=8 -> 1 element per core, zero
collectives). Residual stream kept transposed in SBUF (x^T [768, 1024] f32),
so every matmul contracts along partitions with no PE transposes. All matmuls
in bf16 (1 cyc/row); stats/PSUM/residual in f32. Softmax without
max-subtraction (logits tiny; exp evaluated in f32 on ACT), denominators via a
ones-column appended to V and reciprocal_approx_fast.
"""
import numpy as np
import ml_dtypes
from contextlib import ExitStack

import concourse.bass as bass
import concourse.bacc as bacc
import concourse.tile as tile
import concourse.mybir as mybir
from concourse.bass_utils import run_bass_kernel_spmd

F32 = mybir.dt.float32
BF16 = mybir.dt.bfloat16
I32 = mybir.dt.int32
AF = mybir.ActivationFunctionType
ALU = mybir.AluOpType
BF = ml_dtypes.bfloat16

B, S, D, H, L = 8, 1024, 768, 12, 8
DH, V, COND, FF = 64, 1024, 256, 3072
NT, ND, NV, NF = S // 128, D // 128, V // 128, FF // 128  # 8, 6, 8, 24
FQ = FF // 4  # 768
EPS = 1e-5
INV_SQRT_DH = 1.0 / 8.0

_CACHE = {}


def _bcast_ap(row_ap, nparts):
    """Free-dim step-0 replication source AP from a [1, N] row at any partition."""
    return bass.AP(row_ap.tensor, row_ap.offset,
                   [row_ap.ap[0], [0, nparts]] + row_ap.ap[1:])


def _build():
    nc = bacc.Bacc("TRN2", target_bir_lowering=False, debug=False)

    tok_e = nc.dram_tensor("tok", [1, S], F32, kind="ExternalInput")
    av_e = nc.dram_tensor("avec", [128, 2], BF16, kind="ExternalInput")
    emb_e = nc.dram_tensor("embw", [128, NV, D], BF16, kind="ExternalInput")
    pet_e = nc.dram_tensor("pet", [128, ND, S], F32, kind="ExternalInput")
    lnw_e = nc.dram_tensor("lnw", [L, 128, 8, D], BF16, kind="ExternalInput")
    lnf_e = nc.dram_tensor("lnf", [128, 4, D], BF16, kind="ExternalInput")
    wqkv_e = nc.dram_tensor("wqkv", [L, 128, 18, DH], BF16, kind="ExternalInput")
    wo_e = nc.dram_tensor("wo", [L, 64, H, D], BF16, kind="ExternalInput")
    bo_e = nc.dram_tensor("bo", [L, 1, D], BF16, kind="ExternalInput")
    w1_e = nc.dram_tensor("w1", [L, 128, ND, FF], BF16, kind="ExternalInput")
    b1_e = nc.dram_tensor("b1c", [L, 128, NF], F32, kind="ExternalInput")
    w2_e = nc.dram_tensor("w2", [L, 128, NF, D], BF16, kind="ExternalInput")
    b2_e = nc.dram_tensor("b2", [L, 1, D], BF16, kind="ExternalInput")
    wout_e = nc.dram_tensor("wout", [128, ND, V], BF16, kind="ExternalInput")
    bout_e = nc.dram_tensor("bout", [1, V], BF16, kind="ExternalInput")
    out_e = nc.dram_tensor("out", [S, V], F32, kind="ExternalOutput")

    with tile.TileContext(nc) as tc, ExitStack() as ctx:
        con = ctx.enter_context(tc.tile_pool(name="con", bufs=1))
        xp = ctx.enter_context(tc.tile_pool(name="xp", bufs=1))
        hp = ctx.enter_context(tc.tile_pool(name="hp", bufs=1))
        big = ctx.enter_context(tc.tile_pool(name="big", bufs=1))
        wp = ctx.enter_context(tc.tile_pool(name="wp", bufs=1))
        w12 = ctx.enter_context(tc.tile_pool(name="w12", bufs=2))
        sc = ctx.enter_context(tc.tile_pool(name="sc", bufs=2))
        rp = ctx.enter_context(tc.tile_pool(name="rp", bufs=2))
        ps = ctx.enter_context(tc.tile_pool(name="ps", bufs=4, space="PSUM"))

        # ---- constants
        ones_c = con.tile([128, 1], BF16, tag="ones_c")
        nc.vector.memset(ones_c[:], 1.0)
        ones_r = con.tile([1, 512], BF16, tag="ones_r")
        nc.vector.memset(ones_r[:], 1.0)
        ones_r128 = con.tile([1, 128], BF16, tag="ones_r128")
        nc.vector.memset(ones_r128[:], 1.0)
        eps_t = con.tile([1, 1], F32, tag="eps_t")
        nc.vector.memset(eps_t[:], EPS)
        vidx = con.tile([128, NV], F32, tag="vidx")
        vidx_i = con.tile([128, 1], I32, tag="vidx_i")
        nc.gpsimd.iota(vidx_i[:], [[0, 1]], base=0, channel_multiplier=1)
        vidx_f = con.tile([128, 1], F32, tag="vidx_f")
        nc.vector.tensor_copy(vidx_f[:], vidx_i[:])
        for vc in range(NV):
            nc.vector.tensor_scalar(vidx[:, vc:vc + 1], vidx_f[:], float(vc * 128), None, ALU.add)
        avec = con.tile([128, 2], BF16, tag="avec")
        nc.sync.dma_start(avec[:], av_e.ap()[:])

        # residual x^T and h^T as per-d-chunk tiles (fine-grained deps)
        xTs = [xp.tile([128, S], F32, tag=f"xT{i}", name=f"xT{i}") for i in range(ND)]
        hTs = [hp.tile([128, S], BF16, tag=f"hT{i}", name=f"hT{i}") for i in range(ND)]

        # ---- embedding: x^T = emb^T @ onehot(tok) + pe^T  (scoped pools, 2 dc-groups)
        with tc.tile_pool(name="ep1", bufs=2) as ep1:
            tok_f = ep1.tile([1, S], F32, tag="tok_f", bufs=1)
            nc.sync.dma_start(tok_f[:], tok_e.ap()[:])
            tokrep = ep1.tile([128, S], F32, tag="tokrep", bufs=1)
            nc.gpsimd.partition_broadcast(tokrep[:], tok_f[:])
            for g in range(2):
                _ptags = ["A", "A", "A", "A", "B", "B"]
                pss = [ps.tile([128, 512], F32, tag=_ptags[i], bufs=(4 if _ptags[i] == "A" else 2),
                               name=f"pss{g}_{i}") for i in range(6)]
                for vc in range(NV):
                    oh = ep1.tile([128, S], BF16, tag="oh", name=f"oh{g}_{vc}")
                    nc.vector.tensor_scalar(oh[:], tokrep[:], vidx[:, vc:vc + 1], None, ALU.is_equal)
                    embq = ep1.tile([128, D], BF16, tag="embq", name=f"embq{g}_{vc}")
                    nc.sync.dma_start(embq[:], emb_e.ap()[:, vc, :])
                    for dc3 in range(3):
                        dc = g * 3 + dc3
                        for nh in range(2):
                            nc.tensor.matmul(pss[dc3 * 2 + nh][:],
                                             embq[:, dc * 128: dc * 128 + 128],
                                             oh[:, nh * 512: nh * 512 + 512],
                                             start=(vc == 0), stop=(vc == NV - 1))
                for dc3 in range(3):
                    dc = g * 3 + dc3
                    pet = ep1.tile([128, S], F32, tag="pet", bufs=1, name=f"pet{dc}")
                    nc.sync.dma_start(pet[:], pet_e.ap()[:, dc, :])
                    for nh in range(2):
                        nc.vector.tensor_tensor(xTs[dc][:, nh * 512: nh * 512 + 512],
                                                pss[dc3 * 2 + nh][:], pet[:, nh * 512: nh * 512 + 512], ALU.add)

        def cond_vectors(lnw_tile, nvec, gc_tag):
            """gcols [128, ND*nvec] f32: column (dc*nvec + m) = vec m, d-chunk dc."""
            gcols = sc.tile([128, ND * nvec], F32, tag=gc_tag, name=gc_tag)
            for dc in range(ND):
                gv = ps.tile([128, nvec], F32, tag="C", bufs=2, name=f"gv{dc}")
                for m in range(nvec):
                    for kc in range(2):
                        nc.tensor.matmul(gv[:, m:m + 1],
                                         lnw_tile[:, (m * 2 + kc) * D + dc * 128: (m * 2 + kc) * D + dc * 128 + 128],
                                         avec[:, kc:kc + 1],
                                         start=(kc == 0), stop=(kc == 1))
                nc.scalar.copy(gcols[:, dc * nvec: dc * nvec + nvec], gv[:])
            return gcols

        def cond_ln(srcs, dsts, gcols, gi, bi, nvec):
            """dsts (bf16 chunks) = condLN(srcs f32 chunks); gain/bias gcols cols dc*nvec+{gi,bi}."""
            # running sums over chunks (each xb/x2 dies right after its add)
            xsum = sc.tile([128, S], BF16, tag="xsum", bufs=1, name="xsum")
            x2sum = sc.tile([128, S], BF16, tag="x2sum", bufs=1, name="x2sum")
            for dc in range(ND):
                xb = sc.tile([128, S], BF16, tag="xb", name=f"xb{dc}")
                nc.vector.tensor_scalar(xb[:], srcs[dc][:], 1.0, None, ALU.mult)
                x2 = sc.tile([128, S], BF16, tag="x2", name=f"x2{dc}")
                nc.vector.tensor_tensor(x2[:], xb[:], xb[:], ALU.mult)
                if dc == 0:
                    nc.vector.tensor_copy(xsum[:], xb[:])
                    nc.vector.tensor_copy(x2sum[:], x2[:])
                else:
                    nc.vector.tensor_tensor(xsum[:], xsum[:], xb[:], ALU.add)
                    nc.vector.tensor_tensor(x2sum[:], x2sum[:], x2[:], ALU.add)
            sx = [ps.tile([1, 512], F32, tag="A", bufs=4, name=f"sx{i}") for i in range(2)]
            sxx = [ps.tile([1, 512], F32, tag="A", bufs=4, name=f"sxx{i}") for i in range(2)]
            for nh in range(2):
                nc.tensor.matmul(sx[nh][:], ones_c[:], xsum[:, nh * 512: nh * 512 + 512],
                                 start=True, stop=True)
                nc.tensor.matmul(sxx[nh][:], ones_c[:], x2sum[:, nh * 512: nh * 512 + 512],
                                 start=True, stop=True)
            mu = sc.tile([1, S], F32, tag="rowA", bufs=1, name="mu")
            var = sc.tile([1, S], F32, tag="rowB", bufs=1, name="var")
            for nh in range(2):
                sl = slice(nh * 512, nh * 512 + 512)
                nc.vector.tensor_scalar(mu[:, sl], sx[nh][:], 1.0 / D, None, ALU.mult)
                nc.vector.tensor_scalar(var[:, sl], sxx[nh][:], 1.0 / D, None, ALU.mult)
            murep = rp.tile([128, S], F32, tag="rep", name="murep")
            nc.gpsimd.partition_broadcast(murep[:], mu[:])
            tmp = sc.tile([1, S], F32, tag="rowC", bufs=1, name="mu2")
            nc.vector.tensor_tensor(tmp[:], mu[:], mu[:], ALU.mult)
            nc.vector.tensor_tensor(var[:], var[:], tmp[:], ALU.subtract)
            sd = sc.tile([1, S], F32, tag="rowC", bufs=1, name="sd")
            nc.scalar.activation(sd[:], var[:], AF.Sqrt, bias=eps_t[:], scale=1.0)
            rr = sc.tile([1, S], F32, tag="rowA", bufs=1, name="rr")
            nc.vector.reciprocal_approx_fast(rr[:], sd[:])
            rrep = rp.tile([128, S], F32, tag="rep", name="rrep")
            nc.gpsimd.partition_broadcast(rrep[:], rr[:])
            for dc in range(ND):
                t = sc.tile([128, S], BF16, tag="x2", name=f"lnt{dc}")
                nc.vector.tensor_tensor(t[:], srcs[dc][:], murep[:], ALU.subtract)
                t2 = sc.tile([128, S], BF16, tag="xb", name=f"lnt2_{dc}")
                nc.vector.scalar_tensor_tensor(t2[:], t[:], gcols[:, dc * nvec + gi: dc * nvec + gi + 1],
                                               rrep[:], ALU.mult, ALU.mult)
                nc.vector.tensor_scalar(dsts[dc][:], t2[:],
                                        gcols[:, dc * nvec + bi: dc * nvec + bi + 1], None, ALU.add)

        # =============== layers ===============
        for l in range(L):
            lnw = wp.tile([128, 8 * D], BF16, tag="lnw", name=f"lnw{l}")
            nc.sync.dma_start(lnw[:], lnw_e.ap()[l])
            gcols = cond_vectors(lnw, 4, "gcols")

            wqkv = wp.tile([128, 18 * DH], BF16, tag="wqkv", name=f"wqkv{l}")
            nc.sync.dma_start(wqkv[:], wqkv_e.ap()[l])
            wo = wp.tile([64, H * D], BF16, tag="wo", name=f"wo{l}")
            nc.sync.dma_start(wo[:], wo_e.ap()[l])
            bo = wp.tile([1, D], BF16, tag="bo", name=f"bo{l}")
            nc.sync.dma_start(bo[:], bo_e.ap()[l])
            b1c = wp.tile([128, NF], F32, tag="b1c", name=f"b1c{l}")
            nc.sync.dma_start(b1c[:], b1_e.ap()[l])
            b2 = wp.tile([1, D], BF16, tag="b2", name=f"b2{l}")
            nc.sync.dma_start(b2[:], b2_e.ap()[l])

            # ---- LN1 -> hT
            cond_ln(xTs, hTs, gcols, 0, 1, 4)

            # ---- attention
            o_all = big.tile([64, H * S], BF16, tag="big", name=f"oall{l}")
            sall = sc.tile([12, S], F32, tag="sall", bufs=1, name=f"sall{l}")
            for h in range(H):
                base = 64 * (h % 2)
                hsl = hTs[h // 2][base:base + 64, :]

                def wsl(s):
                    c = (s * 6 + h // 2) * DH
                    return wqkv[base:base + 64, c:c + DH]

                qb = sc.tile([64, S], BF16, tag="qb", name=f"qb{h}")
                kb = sc.tile([64, S], BF16, tag="kb", name=f"kb{h}")
                for which, dstt in ((0, qb), (1, kb)):
                    for nh in range(2):
                        pq = ps.tile([64, 512], F32, tag="C", bufs=2, name=f"pq{h}_{which}_{nh}")
                        nc.tensor.matmul(pq[:], wsl(which), hsl[:, nh * 512: nh * 512 + 512],
                                         start=True, stop=True)
                        nc.vector.tensor_copy(dstt[:, nh * 512: nh * 512 + 512], pq[:])
                vt = sc.tile([128, NT * 65], BF16, tag="vt", name=f"vt{h}")
                nc.vector.memset(vt[:], 1.0)
                for tc8 in range(NT):
                    pv = ps.tile([128, 64], F32, tag="C", bufs=2, name=f"pv{h}_{tc8}")
                    nc.tensor.matmul(pv[:], hsl[:, tc8 * 128: tc8 * 128 + 128], wsl(2),
                                     start=True, stop=True)
                    nc.vector.tensor_copy(vt[:, tc8 * 65: tc8 * 65 + 64], pv[:])
                ops_ = [ps.tile([65, 512], F32, tag="B", bufs=2, name=f"oaug{h}_{i}") for i in range(2)]
                for kc in range(NT):
                    pt = sc.tile([128, S], BF16, tag="pt", name=f"pt{h}_{kc}")
                    for nh in range(2):
                        pl = ps.tile([128, 512], F32, tag="A", bufs=4, name=f"pl{h}_{kc}_{nh}")
                        nc.tensor.matmul(pl[:], kb[:, kc * 128: kc * 128 + 128],
                                         qb[:, nh * 512: nh * 512 + 512], start=True, stop=True)
                        nc.scalar.activation(pt[:, nh * 512: nh * 512 + 512], pl[:],
                                             AF.Exp, bias=0.0, scale=INV_SQRT_DH)
                    for nh in range(2):
                        nc.tensor.matmul(ops_[nh][:], vt[:, kc * 65: kc * 65 + 65],
                                         pt[:, nh * 512: nh * 512 + 512],
                                         start=(kc == 0), stop=(kc == NT - 1))
                s64 = sc.tile([65, S], F32, tag="s64", bufs=1, name=f"s64_{h}")
                for nh in range(2):
                    sl = slice(nh * 512, nh * 512 + 512)
                    nc.scalar.copy(o_all[:, h * S + nh * 512: h * S + nh * 512 + 512], ops_[nh][0:64, :])
                    nc.scalar.copy(s64[64:65, sl], ops_[nh][64:65, :])
                nc.sync.dma_start(sall[h:h + 1, :], s64[64:65, :])
            rall = sall
            nc.vector.reciprocal_approx_fast(rall[:], sall[:])
            for h in range(H):
                rrh = rp.tile([64, S], F32, tag="rep", name=f"rrh{h}")
                nc.sync.dma_start(rrh[:], _bcast_ap(rall[h:h + 1, :], 64))
                nc.vector.tensor_tensor(o_all[:, h * S:(h + 1) * S],
                                        o_all[:, h * S:(h + 1) * S], rrh[:], ALU.mult)
            # y^T accumulate + residual
            for th in range(2):
                for dc in range(ND):
                    py = ps.tile([128, 512], F32, tag="A", bufs=4, name=f"py{th}_{dc}")
                    for h in range(H):
                        nc.tensor.matmul(py[:], wo[:, h * D + dc * 128: h * D + dc * 128 + 128],
                                         o_all[:, h * S + th * 512: h * S + th * 512 + 512],
                                         start=(h == 0), stop=False)
                    nc.tensor.matmul(py[:], bo[:, dc * 128: dc * 128 + 128], ones_r[:],
                                     start=False, stop=True)
                    sl = slice(th * 512, th * 512 + 512)
                    nc.vector.tensor_tensor(xTs[dc][:, sl], xTs[dc][:, sl], py[:], ALU.add)

            # ---- LN2 -> hT
            cond_ln(xTs, hTs, gcols, 2, 3, 4)

            # ---- FFN in four hidden quarters
            for qq in range(4):
                w1q = w12.tile([128, ND * FQ], BF16, tag="w1q", name=f"w1q{l}_{qq}")
                nc.sync.dma_start(w1q[:], w1_e.ap()[l][:, :, qq * FQ:(qq + 1) * FQ])
                gelu = big.tile([128, 6 * S], BF16, tag="big", name=f"gelu{l}_{qq}")
                for mc in range(6):
                    gmc = qq * 6 + mc
                    for nh in range(2):
                        pf = ps.tile([128, 512], F32, tag="A", bufs=4, name=f"pf{qq}_{mc}_{nh}")
                        for kc in range(ND):
                            nc.tensor.matmul(pf[:],
                                             w1q[:, kc * FQ + mc * 128: kc * FQ + mc * 128 + 128],
                                             hTs[kc][:, nh * 512: nh * 512 + 512],
                                             start=(kc == 0), stop=(kc == ND - 1))
                        nc.scalar.activation(gelu[:, mc * S + nh * 512: mc * S + nh * 512 + 512],
                                             pf[:], AF.Gelu_apprx_tanh,
                                             bias=b1c[:, gmc:gmc + 1], scale=1.0)
                w2q = w12.tile([128, 6 * D], BF16, tag="w2q", name=f"w2q{l}_{qq}")
                nc.sync.dma_start(w2q[:], w2_e.ap()[l][:, qq * 6:(qq + 1) * 6, :])
                for th in range(2):
                    for dc in range(ND):
                        py = ps.tile([128, 512], F32, tag="B", bufs=2, name=f"py2_{qq}_{th}_{dc}")
                        for kc in range(6):
                            nc.tensor.matmul(py[:], w2q[:, kc * D + dc * 128: kc * D + dc * 128 + 128],
                                             gelu[:, kc * S + th * 512: kc * S + th * 512 + 512],
                                             start=(kc == 0), stop=(kc == 5 and qq != 3))
                        if qq == 3:
                            nc.tensor.matmul(py[:], b2[:, dc * 128: dc * 128 + 128], ones_r[:],
                                             start=False, stop=True)
                        sl = slice(th * 512, th * 512 + 512)
                        nc.vector.tensor_tensor(xTs[dc][:, sl], xTs[dc][:, sl], py[:], ALU.add)

        # =============== final LN + unembed ===============
        lnf = wp.tile([128, 4 * D], BF16, tag="lnw", name="lnf")
        nc.sync.dma_start(lnf[:], lnf_e.ap()[:])
        gcf = cond_vectors(lnf, 2, "gcf")
        cond_ln(xTs, hTs, gcf, 0, 1, 2)

        woutt = big.tile([128, ND * V], BF16, tag="big", name="woutt")
        nc.sync.dma_start(woutt[:], wout_e.ap()[:])
        boutt = wp.tile([1, V], BF16, tag="bo", name="boutt")
        nc.sync.dma_start(boutt[:], bout_e.ap()[:])
        for tc8 in range(NT):
            osb = sc.tile([128, V], F32, tag="s64", bufs=1, name=f"osb{tc8}")
            for nh in range(2):
                po = ps.tile([128, 512], F32, tag="C", bufs=2, name=f"po{tc8}_{nh}")
                for dc in range(ND):
                    nc.tensor.matmul(po[:], hTs[dc][:, tc8 * 128: tc8 * 128 + 128],
                                     woutt[:, dc * V + nh * 512: dc * V + nh * 512 + 512],
                                     start=(dc == 0), stop=False)
                nc.tensor.matmul(po[:], ones_r128[:], boutt[:, nh * 512: nh * 512 + 512],
                                 start=False, stop=True)
                nc.vector.tensor_copy(osb[:, nh * 512: nh * 512 + 512], po[:])
            nc.sync.dma_start(out_e.ap()[tc8 * 128:(tc8 + 1) * 128, :], osb[:])

    nc.compile()
    return nc


def _sinusoidal_pe():
    pos = np.arange(S, dtype=np.float32)[:, None]
    div = np.exp(np.arange(0, D, 2, dtype=np.float32) * (-np.log(10000.0) / D))
    pe = np.zeros((S, D), dtype=np.float32)
    pe[:, 0::2] = np.sin(pos * div)
    pe[:, 1::2] = np.cos(pos * div)
    return pe


def _pack_weights(ins):
    """Host-side packing into partition-major SBUF images."""
    w = {}
    emb = np.asarray(ins["emb"], np.float32)
    w["embw"] = np.ascontiguousarray(emb.reshape(NV, 128, D).transpose(1, 0, 2)).astype(BF)
    w["pet"] = np.ascontiguousarray(
        (_sinusoidal_pe().T).reshape(ND, 128, S).transpose(1, 0, 2)).astype(np.float32)

    lnw = np.zeros((L, 128, 8, D), np.float32)
    for l in range(L):
        for m, name in enumerate(["ln1_g", "ln1_b", "ln2_g", "ln2_b"]):
            mat = np.asarray(ins[name], np.float32)[l]  # [COND, D]
            lnw[l, :, m * 2 + 0, :] = mat[0:128]
            lnw[l, :, m * 2 + 1, :] = mat[128:256]
    w["lnw"] = lnw.astype(BF)
    lnf = np.zeros((128, 4, D), np.float32)
    for m, name in enumerate(["lnf_g", "lnf_b"]):
        mat = np.asarray(ins[name], np.float32)
        lnf[:, m * 2 + 0, :] = mat[0:128]
        lnf[:, m * 2 + 1, :] = mat[128:256]
    w["lnf"] = lnf.astype(BF)

    wqkv = np.zeros((L, 128, 18, DH), np.float32)
    for l in range(L):
        for s, name in enumerate(["Wq", "Wk", "Wv"]):
            mat = np.asarray(ins[name], np.float32)[l]  # [H, DH, DH]
            for h in range(H):
                wqkv[l, 64 * (h % 2):64 * (h % 2) + 64, s * 6 + h // 2, :] = mat[h]
    w["wqkv"] = wqkv.astype(BF)

    wo = np.zeros((L, 64, H, D), np.float32)
    for l in range(L):
        mat = np.asarray(ins["Wo"], np.float32)[l]
        for h in range(H):
            wo[l, :, h, :] = mat[64 * h:64 * h + 64, :]
    w["wo"] = wo.astype(BF)
    w["bo"] = np.asarray(ins["bo"], np.float32).reshape(L, 1, D).astype(BF)
    w["w1"] = np.ascontiguousarray(
        np.asarray(ins["W1"], np.float32).reshape(L, ND, 128, FF).transpose(0, 2, 1, 3)).astype(BF)
    w["b1c"] = np.ascontiguousarray(
        np.asarray(ins["b1"], np.float32).reshape(L, NF, 128).transpose(0, 2, 1)).astype(np.float32)
    w["w2"] = np.ascontiguousarray(
        np.asarray(ins["W2"], np.float32).reshape(L, NF, 128, D).transpose(0, 2, 1, 3)).astype(BF)
    w["b2"] = np.asarray(ins["b2"], np.float32).reshape(L, 1, D).astype(BF)
    w["wout"] = np.ascontiguousarray(
        np.asarray(ins["Wout"], np.float32).reshape(ND, 128, V).transpose(1, 0, 2)).astype(BF)
    w["bout"] = np.asarray(ins["bout"], np.float32).reshape(1, V).astype(BF)
    return w


def kernel(**inputs):
    if "nc" not in _CACHE:
        _CACHE["nc"] = _build()
    nc = _CACHE["nc"]

    w = _pack_weights(inputs)
    tokens = np.asarray(inputs["tokens"]).astype(np.int32)      # [B, S]
    actions = np.asarray(inputs["actions"]).astype(np.int64)    # [B, NACT]
    act_emb = np.asarray(inputs["act_emb"], np.float32)         # [NCOND, DPA]

    in_maps = []
    for b in range(B):
        a = act_emb[actions[b]].reshape(COND).astype(np.float32)  # [256]
        av = np.ascontiguousarray(a.reshape(2, 128).T).astype(BF)  # [128, 2]
        m = {"tok": tokens[b:b + 1].astype(np.float32), "avec": av}
        m.update(w)
        in_maps.append(m)

    res = run_bass_kernel_spmd(nc, in_maps, core_ids=list(range(B)))
    out = np.stack([res.results[b]["out"] for b in range(B)], axis=0)
    return out.astype(np.float32)
